# revision 14
# baseline (speedup 1.0000x reference)
"""Multi-head self-attention (B=4, S=2048, D=768, H=12, dh=64) on 8 trn2 cores.

Sharding: core = b*2 + g  (b = batch 0..3, g = head-group of 6 heads).
Each core computes q/k/v projections for its 6 heads over the full sequence,
masked softmax attention, and a partial output projection (column slice of
o_w => row-parallel). Host sums the two partial outputs per batch element.

v2 design (ACT-exp is the ~115us floor; everything else hides under it):
  - mask gather on host: only unmasked k positions (padded to a multiple of
    128) are shipped/projected/exp'd; pad columns get a -1e30 per-partition
    bias inside the ACT exp instruction (out = exp(scale*s + bias)).
  - scoresT [kv, q] layout; per head the score contraction is K=64 (dh), so
    the two heads of a pair run as CONCURRENT row-tiled matmuls
    (tile_position (0,0) and (64,0)) into separate psum banks - no zero
    padding, up to 2x score throughput.
  - exp on ACT in [128, 1024] tiles; exp tiles persist in SBUF (22-slot
    rotation) so the q-half ctx units can consume them on their own schedule.
  - ctx matmul per head has a 65th 'ones' column in v (from an interleaved
    augmented wv built on host) whose psum row 64 gives the softmax sums for
    free.  ctx runs as two 512-wide halves: half0 per kc inside the unit,
    half1 as a burst after the unit (keeps only one half-unit per psum bank).
  - tails: DVE approx-reciprocal of the sums row, GpSimd partition_broadcast
    of the reciprocal across 64 partitions, one DVE multiply into ctx_sb.
  - psum budget (8 banks): scores A+B = 4, ctx A+B = 2, 2 filler banks for
    q/k/v/out projections paced inside the ACT-bound attention stream.
"""

from collections import deque

import numpy as np
import ml_dtypes

import concourse.bass as bass
import concourse.mybir as mybir
import concourse.tile as tile
from concourse import bacc
from concourse.bass_utils import run_bass_kernel_spmd

BS, SEQ, DIM, NH = 4, 2048, 768, 12
DH = 64
HEADS = 6            # heads per core
NPAIR = 3            # head-pairs per core
DGRP = HEADS * DH    # 384
VGRP = HEADS * 65    # 390 (65-interleaved: 64 ctx cols + ones col per head)
N_CORES = 8
P = 128
QH = 1024            # exp/scores tile width (q)
NT = SEQ // 512      # 4

F32 = mybir.dt.float32
BF16 = mybir.dt.bfloat16

MM_DT = BF16
MM_NP = ml_dtypes.bfloat16 if MM_DT == BF16 else np.float32

NEG = -1.0e30
KIN = DIM // P       # 6 contraction chunks for q/k proj
KIN_V = 7            # 768 inputs + ones row, padded to 896


def _build(NKV: int):
    """Build the per-core Bass program, parameterized by padded kv length."""
    KC = NKV // P          # kv chunks
    QC = SEQ // P          # 16 q chunks for out proj

    nc = bacc.Bacc(None, target_bir_lowering=False, debug=False)

    xqT = nc.declare_dram_parameter("xqT", [DIM, SEQ], MM_DT, isOutput=False)
    xkT = nc.declare_dram_parameter("xkT", [DIM, NKV], MM_DT, isOutput=False)
    xvT = nc.declare_dram_parameter("xvT", [P * KIN_V, NKV], MM_DT, isOutput=False)
    wqT = nc.declare_dram_parameter("wqT", [DIM, DGRP], MM_DT, isOutput=False)
    wkT = nc.declare_dram_parameter("wkT", [DIM, DGRP], MM_DT, isOutput=False)
    wvT = nc.declare_dram_parameter("wvT", [P * KIN_V, VGRP], MM_DT, isOutput=False)
    woT = nc.declare_dram_parameter("woT", [DGRP, DIM], MM_DT, isOutput=False)
    qb = nc.declare_dram_parameter("qb", [DGRP], F32, isOutput=False)
    kb = nc.declare_dram_parameter("kb", [DGRP], F32, isOutput=False)
    ob = nc.declare_dram_parameter("ob", [DIM], F32, isOutput=False)
    pb = nc.declare_dram_parameter("pb", [NKV], F32, isOutput=False)
    out = nc.declare_dram_parameter("out", [SEQ, DIM], F32, isOutput=True)

    xqT_r = xqT.rearrange("(kk pi) n -> pi kk n", pi=P)
    xkT_r = xkT.rearrange("(kk pi) n -> pi kk n", pi=P)
    xvT_r = xvT.rearrange("(kk pi) n -> pi kk n", pi=P)
    wqT_r = wqT.rearrange("(kk pi) n -> pi kk n", pi=P)
    wkT_r = wkT.rearrange("(kk pi) n -> pi kk n", pi=P)
    wvT_r = wvT.rearrange("(kk pi) n -> pi kk n", pi=P)
    woT_r = woT.rearrange("(kk pi) n -> pi kk n", pi=P)
    qb_r = qb.rearrange("(m pi) -> pi m", pi=P)
    kb_r = kb.rearrange("(m pi) -> pi m", pi=P)
    pb_r = pb.rearrange("(c pi) -> pi c", pi=P)

    # k-proj column slices (along kv)
    ksl = []
    o = 0
    while o < NKV:
        w = min(512, NKV - o)
        ksl.append((o, w))
        o += w

    with tile.TileContext(nc) as tc:
        with (
            tc.tile_pool(name="const", bufs=1) as const,
            tc.tile_pool(name="stage", bufs=1) as stage,
            tc.tile_pool(name="persist", bufs=1) as persist,
            tc.tile_pool(name="expp", bufs=1) as expp,
            tc.tile_pool(name="outp", bufs=1) as outp,
            tc.tile_pool(name="stat", bufs=1) as stat,
            tc.tile_pool(name="psS", bufs=1, space="PSUM") as psS,
            tc.tile_pool(name="psC", bufs=1, space="PSUM") as psC,
            tc.tile_pool(name="psF", bufs=1, space="PSUM") as psF,
        ):
            # ---- constants / weights (DMA first; k-proj path is critical) ----
            pb_sb = const.tile([P, KC], F32, name="pb_sb")
            nc.sync.dma_start(pb_sb[:], pb_r)
            qb_sb = const.tile([P, NPAIR], F32, name="qb_sb")
            nc.sync.dma_start(qb_sb[:], qb_r)
            kb_sb = const.tile([P, NPAIR], F32, name="kb_sb")
            nc.sync.dma_start(kb_sb[:], kb_r)
            ob_row = const.tile([1, DIM], F32, name="ob_row")
            nc.sync.dma_start(ob_row[:], ob[None, :])
            wk_sb = const.tile([P, KIN, DGRP], MM_DT, name="wk_sb")
            for kk in range(KIN):
                nc.sync.dma_start(wk_sb[:, kk, :], wkT_r[:, kk, :])
            wq_sb = const.tile([P, KIN, DGRP], MM_DT, name="wq_sb")
            for kk in range(KIN):
                nc.sync.dma_start(wq_sb[:, kk, :], wqT_r[:, kk, :])


            # ---- input staging: split DMAs into ~100-200KB pieces spread
            # across the 16 queues, critical-path pieces first ----
            xk_sb = stage.tile([P, KIN, NKV], MM_DT, name="xk_sb")
            for o0, w in ksl:
                for kk in range(KIN):
                    nc.sync.dma_start(xk_sb[:, kk, o0:o0 + w],
                                      xkT_r[:, kk, o0:o0 + w])
            xq_sb = stage.tile([P, KIN, SEQ], MM_DT, name="xq_sb")
            for nt in range(2):
                for kk in range(KIN):
                    nc.sync.dma_start(xq_sb[:, kk, nt * 512:(nt + 1) * 512],
                                      xqT_r[:, kk, nt * 512:(nt + 1) * 512])
            wv_sb = const.tile([P, KIN_V, VGRP], MM_DT, name="wv_sb")
            for kk in range(KIN_V):
                nc.sync.dma_start(wv_sb[:, kk, :], wvT_r[:, kk, :])
            xv_sb = stage.tile([P, KIN_V, NKV], MM_DT, name="xv_sb")
            for o0, w in ksl:
                for kk in range(KIN_V):
                    nc.sync.dma_start(xv_sb[:, kk, o0:o0 + w],
                                      xvT_r[:, kk, o0:o0 + w])
            wo_sb = const.tile([P, NPAIR, DIM], MM_DT, name="wo_sb")
            for kk in range(NPAIR):
                nc.sync.dma_start(wo_sb[:, kk, :], woT_r[:, kk, :])
            for nt in range(2, NT):
                for kk in range(KIN):
                    nc.sync.dma_start(xq_sb[:, kk, nt * 512:(nt + 1) * 512],
                                      xqT_r[:, kk, nt * 512:(nt + 1) * 512])

            # o_b broadcast across partitions (GpSimd; PE-free)
            ob_bc = const.tile([P, DIM], F32, name="ob_bc")
            nc.gpsimd.partition_broadcast(ob_bc[:], ob_row[:])

            # ---- persistent activations ----
            qT_sb = persist.tile([P, NPAIR, SEQ], MM_DT, name="qT_sb")
            kT_sb = persist.tile([P, NPAIR, NKV], MM_DT, name="kT_sb")
            v_sb = persist.tile([P, KC, VGRP], MM_DT, name="v_sb")
            ctx_sb = persist.tile([P, NPAIR, SEQ], MM_DT, name="ctx_sb")

            fctr = [0]

            def vproj_chunk(m):
                fctr[0] += 1
                ps = psF.tile([P, 512], F32, tag=f"f{fctr[0] % 2}",
                              name="psv", bufs=1)
                for kk in range(KIN_V):
                    nc.tensor.matmul(
                        ps[:, 0:VGRP],
                        xv_sb[:, kk, m * P:(m + 1) * P],
                        wv_sb[:, kk, :],
                        start=(kk == 0), stop=(kk == KIN_V - 1),
                    )
                nc.vector.tensor_copy(out=v_sb[:, m, :], in_=ps[:, 0:VGRP])

            def qproj_chunk(p, nt):
                fctr[0] += 1
                ps = psF.tile([P, 512], F32, tag=f"f{fctr[0] % 2}",
                              name="psq", bufs=1)
                for kk in range(KIN):
                    nc.tensor.matmul(
                        ps[:],
                        wq_sb[:, kk, p * P:(p + 1) * P],
                        xq_sb[:, kk, nt * 512:(nt + 1) * 512],
                        start=(kk == 0), stop=(kk == KIN - 1),
                    )
                nc.vector.tensor_scalar_add(
                    qT_sb[:, p, nt * 512:(nt + 1) * 512], ps[:],
                    qb_sb[:, p, None],
                )

            def kproj_chunk(p, si):
                fctr[0] += 1
                o0, w = ksl[si]
                ps = psF.tile([P, 512], F32, tag=f"f{fctr[0] % 2}",
                              name="psk", bufs=1)
                for kk in range(KIN):
                    nc.tensor.matmul(
                        ps[:, 0:w],
                        wk_sb[:, kk, p * P:(p + 1) * P],
                        xk_sb[:, kk, o0:o0 + w],
                        start=(kk == 0), stop=(kk == KIN - 1),
                    )
                nc.vector.tensor_scalar_add(
                    kT_sb[:, p, o0:o0 + w], ps[:, 0:w],
                    kb_sb[:, p, None],
                )

            def outproj_chunk(qc):
                o_t = outp.tile([P, DIM], F32, tag="o", name="o_t", bufs=3)
                for n0 in (0, 384):
                    fctr[0] += 1
                    ps = psF.tile([P, 512], F32, tag=f"f{fctr[0] % 2}",
                                  name="pso", bufs=1)
                    for kk in range(NPAIR):
                        nc.tensor.matmul(
                            ps[:, 0:384],
                            ctx_sb[:, kk, qc * P:(qc + 1) * P],
                            wo_sb[:, kk, n0:n0 + 384],
                            start=(kk == 0), stop=(kk == NPAIR - 1),
                        )
                    nc.vector.tensor_tensor(
                        o_t[:, n0:n0 + 384], ps[:, 0:384],
                        ob_bc[:, n0:n0 + 384],
                        mybir.AluOpType.add,
                    )
                nc.sync.dma_start(out[qc * P:(qc + 1) * P, :], o_t[:])

            # ---- prefix: minimal critical path for unit 0 ----
            for si in range(len(ksl)):
                kproj_chunk(0, si)
            qproj_chunk(0, 0)
            qproj_chunk(0, 1)
            vproj_chunk(0)
            vproj_chunk(1)

            # ---- filler queues, paced inside the attention stream ----
            Qv = deque(range(2, KC))
            Qproj = deque(
                [(1, lambda si=si: kproj_chunk(1, si)) for si in range(len(ksl))]
                + [(1, lambda: qproj_chunk(1, 0)), (1, lambda: qproj_chunk(1, 1))]
                + [(2, lambda si=si: kproj_chunk(2, si)) for si in range(len(ksl))]
                + [(2, lambda: qproj_chunk(2, 0)), (2, lambda: qproj_chunk(2, 1))]
                + [(3, lambda: qproj_chunk(0, 2)), (3, lambda: qproj_chunk(0, 3)),
                   (4, lambda: qproj_chunk(1, 2)), (4, lambda: qproj_chunk(1, 3)),
                   (5, lambda: qproj_chunk(2, 2)), (5, lambda: qproj_chunk(2, 3))]
            )
            Qout = deque()

            def pace_fillers(u, kc):
                # keep v-proj 4 chunks ahead of (lagged) ctx consumption
                while Qv and Qv[0] <= kc + 2:
                    vproj_chunk(Qv.popleft())
                if Qproj:
                    Qproj.popleft()[1]()
                elif Qout and kc % 2 == 1:
                    outproj_chunk(Qout.popleft())

            def flush_due(u):
                while Qproj and Qproj[0][0] <= u:
                    Qproj.popleft()[1]()

            # ---- attention ----
            units = [(qh, p) for qh in range(SEQ // QH) for p in range(NPAIR)]
            exp_tiles = {}
            LAG = 2

            def tail(p, qh, half, h, ps_ctx):
                q0 = qh * QH + half * 512
                sums_t = stat.tile([1, 512], F32, tag="s", name="sums_t",
                                   bufs=4)
                nc.vector.tensor_copy(out=sums_t[:], in_=ps_ctx[64:65, :])
                recip_t = stat.tile([1, 512], F32, tag="r", name="recip_t",
                                    bufs=4)
                nc.vector.reciprocal_approx_fast(
                    out=recip_t[:], in_=sums_t[:])
                rbc = stat.tile([DH, 512], F32, tag="rb", name="rbc", bufs=4)
                nc.gpsimd.partition_broadcast(rbc[:], recip_t[:])
                nc.vector.tensor_tensor(
                    ctx_sb[64 * h:64 * h + DH, p, q0:q0 + 512],
                    ps_ctx[0:DH, :],
                    rbc[:],
                    mybir.AluOpType.mult,
                )

            ctx_live = {}

            def ctx_item(u, half, kc):
                qh, p = units[u]
                key = (u, half)
                if key not in ctx_live:
                    ctx_live[key] = (
                        psC.tile([P, 512], F32, tag="cA", name="pscA", bufs=1),
                        psC.tile([P, 512], F32, tag="cB", name="pscB", bufs=1),
                    )
                tiles = ctx_live[key]
                for h in range(2):
                    g = 2 * p + h
                    nc.tensor.matmul(
                        tiles[h][0:65, :],
                        v_sb[:, kc, 65 * g:65 * g + 65],
                        exp_tiles[(u, h, kc)][:, half * 512:(half + 1) * 512],
                        start=(kc == 0), stop=(kc == KC - 1),
                    )
                    if half == 1:
                        exp_tiles.pop((u, h, kc), None)
                if kc == KC - 1:
                    for h in range(2):
                        tail(p, qh, half, h, tiles[h])
                    del ctx_live[key]

            ctxq = deque()

            def pop_ctx(nmax, keep=LAG):
                n = 0
                while ctxq and len(ctxq) > keep and n < nmax:
                    ctx_item(*ctxq.popleft())
                    n += 1

            def emit_scores_exp(u, kc):
                qh, p = units[u]
                ps_s = [psS.tile([P, QH], F32, tag="sA", name="pssA", bufs=1),
                        psS.tile([P, QH], F32, tag="sB", name="pssB", bufs=1)]
                for qt in range(QH // 512):
                    c0 = qh * QH + qt * 512
                    for h in range(2):
                        nc.tensor.matmul(
                            ps_s[h][:, qt * 512:(qt + 1) * 512],
                            kT_sb[64 * h:64 * (h + 1), p,
                                  kc * P:(kc + 1) * P],
                            qT_sb[64 * h:64 * (h + 1), p, c0:c0 + 512],
                            start=True, stop=True,
                            tile_position=(64 * h, 0),
                        )
                for h in range(2):
                    e_t = expp.tile([P, QH], MM_DT, tag="e", name="e_t",
                                    bufs=22)
                    nc.scalar.activation(
                        e_t[:], ps_s[h][:],
                        mybir.ActivationFunctionType.Exp,
                        bias=pb_sb[:, kc, None], scale=0.125,
                    )
                    exp_tiles[(u, h, kc)] = e_t

            # software-pipelined: scores+exp for step kc+1 (cross-unit at
            # boundaries) are emitted BEFORE step kc's ctx pops and fillers,
            # keeping the ACT critical path ahead in PE priority.
            emit_scores_exp(0, 0)
            for u, (qh, p) in enumerate(units):
                flush_due(u)
                for kc in range(KC):
                    if kc + 1 < KC:
                        emit_scores_exp(u, kc + 1)
                    elif u + 1 < len(units):
                        emit_scores_exp(u + 1, 0)
                    ctxq.append((u, 0, kc))
                    pop_ctx(3)
                    pace_fillers(u, kc)
                for kc in range(KC):
                    ctxq.append((u, 1, kc))
                if p == NPAIR - 1:
                    Qout.extend(range(qh * (QC // 2), (qh + 1) * (QC // 2)))
            while ctxq:
                ctx_item(*ctxq.popleft())

            # ---- suffix: flush remaining fillers and out-proj ----
            while Qv:
                vproj_chunk(Qv.popleft())
            while Qproj:
                Qproj.popleft()[1]()
            while Qout:
                outproj_chunk(Qout.popleft())

    nc.compile()
    return nc


_cache: dict = {}

# test harnesses may set e.g. {"trace": True, "tmpdir": ...}; empty for grading
_run_opts: dict = {}
LAST_RES = None


def _get_nc(NKV: int):
    if NKV not in _cache:
        _cache[NKV] = _build(NKV)
    return _cache[NKV]


def kernel(query, key_, value, mask, q_w, q_b, k_w, k_b, v_w, v_b, o_w, o_b):
    query = np.asarray(query, np.float32)
    key_ = np.asarray(key_, np.float32)
    value = np.asarray(value, np.float32)
    mask = np.asarray(mask)
    q_w = np.asarray(q_w, np.float32)
    q_b = np.asarray(q_b, np.float32)
    k_w = np.asarray(k_w, np.float32)
    k_b = np.asarray(k_b, np.float32)
    v_w = np.asarray(v_w, np.float32)
    v_b = np.asarray(v_b, np.float32)
    o_w = np.asarray(o_w, np.float32)
    o_b = np.asarray(o_b, np.float32)

    counts = (mask != 0).sum(axis=1)
    NKV = max(P, int(-(-int(counts.max()) // P) * P))
    nc = _get_nc(NKV)

    zeros_ob = np.zeros_like(o_b)
    in_maps = []
    for b in range(BS):
        idx = np.nonzero(mask[b])[0]
        cnt = len(idx)
        xk_g = np.zeros((NKV, DIM), np.float32)
        xv_g = np.zeros((NKV, DIM), np.float32)
        xk_g[:cnt] = key_[b][idx]
        xv_g[:cnt] = value[b][idx]
        xqT_b = np.ascontiguousarray(query[b].T).astype(MM_NP)
        xkT_b = np.ascontiguousarray(xk_g.T).astype(MM_NP)
        xvT_b = np.zeros((P * KIN_V, NKV), MM_NP)
        xvT_b[:DIM] = xv_g.T
        xvT_b[DIM] = 1.0
        pb_b = np.where(np.arange(NKV) < cnt, 0.0, NEG).astype(np.float32)
        for g in range(2):
            sl = slice(DGRP * g, DGRP * (g + 1))
            # interleaved augmented wv: col 65h+j (j<64) = v_w.T col, rows
            # 0-767; row 768 = v_b;  col 65h+64 = ones-selector (row 768 = 1).
            wv_aug = np.zeros((P * KIN_V, VGRP), np.float32)
            vwT = v_w[sl].T  # [768, 384]
            vb = v_b[sl]
            for h in range(HEADS):
                wv_aug[:DIM, 65 * h:65 * h + 64] = vwT[:, 64 * h:64 * h + 64]
                wv_aug[DIM, 65 * h:65 * h + 64] = vb[64 * h:64 * h + 64]
                wv_aug[DIM, 65 * h + 64] = 1.0
            in_maps.append({
                "xqT": xqT_b,
                "xkT": xkT_b,
                "xvT": xvT_b,
                "wqT": np.ascontiguousarray(q_w[sl].T).astype(MM_NP),
                "wkT": np.ascontiguousarray(k_w[sl].T).astype(MM_NP),
                "wvT": wv_aug.astype(MM_NP),
                "woT": np.ascontiguousarray(o_w[:, sl].T).astype(MM_NP),
                "qb": q_b[sl].copy(),
                "kb": k_b[sl].copy(),
                "ob": o_b if g == 0 else zeros_ob,
                "pb": pb_b,
            })

    res = run_bass_kernel_spmd(nc, in_maps, core_ids=list(range(N_CORES)),
                               **_run_opts)
    global LAST_RES
    LAST_RES = res
    out = np.empty((BS, SEQ, DIM), np.float32)
    for b in range(BS):
        out[b] = res.results[2 * b]["out"] + res.results[2 * b + 1]["out"]
    return out
